# revision 5
# baseline (speedup 1.0000x reference)
"""Grouped-query attention (B=8,S=512,D=4096,G=32) on 8 trn2 cores.

Strategy: data-parallel over the batch dim — core b handles batch b.
Per core, everything is computed in a feature-major ("transposed")
layout so no on-device transposes are needed:

  q^T[f,t] = sum_d WqT[d,f] * xqT[d,t]        (lhsT=WqT tile, rhs=xqT)
  k^T      likewise;  v[t,f] uses lhsT=xqT tile, rhs=WvT tile
  RoPE on q^T/k^T heads 0..7 (per-token angle, head g pairs with g+4)
  s^T[k,q] = kh^T_blk.T @ qh^T   (per head, 4 k-blocks of 128)
  w^T      = exp(s^T + maskbias) (no max-subtraction; logits are O(10))
  o^T[dh,q]= sum_kb vh_blk.T @ w^T_blk        (lhsT=vh block)
  sum[1,q] = ones.T @ w^T  -> r = 1/sum -> broadcast via rank-1 matmul
  attn^T   = o^T * r_bcast  (bf16)
  y[t,f]   = sum_D attnT_blk.T @ WoT tile

Matmuls run in bf16 (fp32 PSUM accumulation); softmax math in fp32.
Host side only shards, transposes (layout), casts dtypes and gathers.
"""

import math

import numpy as np
import ml_dtypes

import concourse.bass as bass
import concourse.mybir as mybir
import concourse.tile as tile
from concourse import bacc
from concourse.bass_utils import run_bass_kernel_spmd

B, S, D = 8, 512, 4096
G, DH = 32, 128
RD = 1024
ALPHA = 1.0 / math.sqrt(DH)
PI = math.pi
NCORES = 8
DT = mybir.dt
AF = mybir.ActivationFunctionType
ALU = mybir.AluOpType

# set by test.py to capture a profile
TRACE = False
LAST_RESULT = None


def _range_reduce(nc, ang, mtmp):
    """In-place reduce ang (>=0, < ~7*pi/2) into (-pi, pi] mod 2*pi."""
    for _ in range(3):
        # mtmp = (ang > pi) * 2pi ; ang -= mtmp
        nc.vector.tensor_scalar(mtmp, ang, PI, 2.0 * PI, ALU.is_gt, ALU.mult)
        nc.vector.tensor_sub(ang, ang, mtmp)


def build_program():
    # Bacc (not plain Bass): its compile pipeline lowers multi-sem waits to
    # the single ISA wait slot; plain Bass BIR fails walrus codegen.
    nc = bacc.Bacc(
        "TRN2", target_bir_lowering=False, debug=False, num_devices=NCORES
    )
    bf16 = DT.bfloat16
    f32 = DT.float32

    # Weights arrive SHARDED: core c holds rows [c*512, (c+1)*512) of each
    # transposed weight (1/8 of the bytes). An on-chip AllGather rebuilds
    # the full [D, D] matrices in internal DRAM — host->device traffic for
    # weights drops 8x vs replicating them into every core's in_map.
    WSH = D // NCORES  # 512 rows per shard
    xqT_d = nc.declare_dram_parameter("xqT", [D, S], bf16, isOutput=False)
    xkT_d = nc.declare_dram_parameter("xkT", [D, S], bf16, isOutput=False)
    xvT_d = nc.declare_dram_parameter("xvT", [D, S], bf16, isOutput=False)
    wq_sh = nc.declare_dram_parameter("wq_sh", [WSH, D], bf16, isOutput=False)
    wk_sh = nc.declare_dram_parameter("wk_sh", [WSH, D], bf16, isOutput=False)
    wv_sh = nc.declare_dram_parameter("wv_sh", [WSH, D], bf16, isOutput=False)
    wo_sh = nc.declare_dram_parameter("wo_sh", [WSH, D], bf16, isOutput=False)
    pos_d = nc.declare_dram_parameter("pos", [S], f32, isOutput=False)
    invf_d = nc.declare_dram_parameter("invf", [S], f32, isOutput=False)
    mask_d = nc.declare_dram_parameter("maskin", [S], DT.int32, isOutput=False)
    y_d = nc.declare_dram_parameter("y", [S, D], bf16, isOutput=True)

    RG = [list(range(NCORES))]

    with tile.TileContext(nc) as tc:
        with tc.tile_pool(name="dram", bufs=1, space="DRAM") as dram:
            wqT_d = dram.tile([D, D], bf16, addr_space="Shared", tag="wqg")
            wkT_d = dram.tile([D, D], bf16, addr_space="Shared", tag="wkg")
            wvT_d = dram.tile([D, D], bf16, addr_space="Shared", tag="wvg")
            woT_d = dram.tile([D, D], bf16, addr_space="Shared", tag="wog")
            for which, sh_d, g_t in (
                ("q", wq_sh, wqT_d),
                ("k", wk_sh, wkT_d),
                ("v", wv_sh, wvT_d),
                ("o", wo_sh, woT_d),
            ):
                bounce = dram.tile(
                    [WSH, D], bf16, tag=f"b{which}", name=f"bounce_{which}"
                )
                nc.gpsimd.dma_start(out=bounce, in_=sh_d[:])
                nc.gpsimd.collective_compute(
                    "AllGather",
                    ALU.bypass,
                    replica_groups=RG,
                    ins=[bounce.opt()],
                    outs=[g_t.opt()],
                )
            _build_body(nc, tc, xqT_d, xkT_d, xvT_d, wqT_d, wkT_d, wvT_d,
                        woT_d, pos_d, invf_d, mask_d, y_d)
    # Bacc lowering: splits multi-sem waits into the single ISA wait slot,
    # allocates registers, fuses nops. Required before walrus codegen.
    nc.compile()
    return nc


def _build_body(nc, tc, xqT_d, xkT_d, xvT_d, wqT_d, wkT_d, wvT_d, woT_d,
                pos_d, invf_d, mask_d, y_d):
    bf16 = DT.bfloat16
    f32 = DT.float32
    if True:
        with tc.tile_pool(name="persist", bufs=1) as persist:
            # trig tiles, broadcast over partitions: [128, S]
            sin_t = persist.tile([128, S], f32, tag="sin")
            cos_t = persist.tile([128, S], f32, tag="cos")
            sinq_t = persist.tile([128, S], f32, tag="sinq")
            cosq_t = persist.tile([128, S], f32, tag="cosq")
            maskb = persist.tile([128, 4], f32, tag="maskb")
            ones_bf = persist.tile([128, 1], bf16, tag="ones_bf")
            ones_f1 = persist.tile([1, 128], f32, tag="ones_f1")
            qT_s = persist.tile([128, G, S], bf16, tag="qT")
            kT_s = persist.tile([128, G, S], bf16, tag="kT")
            v_s = persist.tile([128, 4, G, DH], bf16, tag="v")

            nc.vector.memset(ones_bf, 1.0)
            nc.vector.memset(ones_f1, 1.0)

            # ---- setup: trig + mask ----
            # Trig is computed on one partition, then broadcast to all 128
            # partitions with a rank-1 matmul (ones [1,128] x row [1,S]).
            with (
                tc.tile_pool(name="setup", bufs=1) as setup,
                tc.tile_pool(name="ps_setup", bufs=2, space="PSUM") as ps_setup,
            ):
                pos1 = setup.tile([1, S], f32, tag="pos1")
                invf1 = setup.tile([1, S], f32, tag="invf1")
                angc = setup.tile([1, S], f32, tag="angc")
                mtmp = setup.tile([1, S], f32, tag="mtmp")
                mi = setup.tile([128, 4], DT.int32, tag="mi")
                mf = setup.tile([128, 4], f32, tag="mf")

                nc.sync.dma_start(out=pos1, in_=pos_d[None, :])
                nc.sync.dma_start(out=invf1, in_=invf_d[None, :])
                # angles for sin, reuse pos1 as buffer for sin-angles
                angs = pos1
                nc.vector.tensor_mul(angs, pos1, invf1)
                nc.vector.tensor_scalar_add(angc, angs, PI / 2.0)
                _range_reduce(nc, angs, mtmp)
                _range_reduce(nc, angc, mtmp)
                nc.scalar.activation(angs, angs, AF.Sin)
                nc.scalar.activation(angc, angc, AF.Sin)
                ps_sin = ps_setup.tile([128, S], f32, tag="b", name="ps_sin")
                ps_cos = ps_setup.tile([128, S], f32, tag="b", name="ps_cos")
                nc.tensor.matmul(ps_sin, ones_f1, angs, start=True, stop=True)
                nc.tensor.matmul(ps_cos, ones_f1, angc, start=True, stop=True)
                nc.scalar.copy(sin_t, ps_sin)
                nc.scalar.copy(cos_t, ps_cos)
                nc.scalar.mul(sinq_t, ps_sin, ALPHA)
                nc.scalar.mul(cosq_t, ps_cos, ALPHA)

                nc.sync.dma_start(out=mi, in_=mask_d[:].rearrange("(b p) -> p b", p=128))
                nc.vector.tensor_copy(mf, mi)
                # maskb = m * 1e9 - 1e9  (0 where m==1, -1e9 where m==0)
                nc.vector.tensor_scalar(
                    maskb, mf, 1.0e9, 1.0e9, ALU.mult, ALU.subtract
                )

            # ---- q and k projections (output feature-major) + RoPE ----
            for which, wT_d, xT_d, outT, ct, st in (
                ("q", wqT_d, xqT_d, qT_s, cosq_t, sinq_t),
                ("k", wkT_d, xkT_d, kT_s, cos_t, sin_t),
            ):
                scale = ALPHA if which == "q" else 1.0
                with (
                    tc.tile_pool(name=f"x{which}", bufs=1) as xin_pool,
                    tc.tile_pool(name=f"w{which}", bufs=12) as w_pool,
                    tc.tile_pool(name=f"ps{which}", bufs=8, space="PSUM") as ps_pool,
                    tc.tile_pool(name=f"rope{which}", bufs=1) as rope_pool,
                    tc.tile_pool(name=f"rt{which}", bufs=8) as rtmp_pool,
                ):
                    xT_s = xin_pool.tile([128, 32, S], bf16, tag="xT")
                    xr = xT_d[:].rearrange("(db p) t -> p db t", p=128)
                    for c in range(4):
                        nc.sync.dma_start(
                            out=xT_s[:, c * 8 : (c + 1) * 8, :],
                            in_=xr[:, c * 8 : (c + 1) * 8, :],
                        )
                    rope_f32 = rope_pool.tile([128, 8, S], f32, tag="rope")
                    for gq in range(8):
                        pss = [
                            ps_pool.tile([128, S], f32, tag="ps", name="ps_qk") for _ in range(4)
                        ]
                        for d in range(32):
                            wt = w_pool.tile([128, 512], bf16, tag="w")
                            nc.sync.dma_start(
                                out=wt,
                                in_=wT_d[
                                    d * 128 : (d + 1) * 128,
                                    gq * 512 : (gq + 1) * 512,
                                ],
                            )
                            for g4 in range(4):
                                nc.tensor.matmul(
                                    pss[g4],
                                    wt[:, g4 * 128 : (g4 + 1) * 128],
                                    xT_s[:, d, :],
                                    start=(d == 0),
                                    stop=(d == 31),
                                )
                        for g4 in range(4):
                            g = gq * 4 + g4
                            if g < 8:
                                # RoPE heads: stash fp32
                                nc.vector.tensor_copy(rope_f32[:, g, :], pss[g4])
                            elif which == "q":
                                nc.scalar.activation(
                                    outT[:, g, :], pss[g4], AF.Copy, scale=scale
                                )
                            else:
                                nc.vector.tensor_copy(outT[:, g, :], pss[g4])
                    # RoPE: head g pairs with head g+4 (per-token scalar angle)
                    for g in range(4):
                        a = rope_f32[:, g, :]
                        b = rope_f32[:, g + 4, :]
                        t1 = rtmp_pool.tile([128, S], f32, tag="t")
                        t2 = rtmp_pool.tile([128, S], f32, tag="t")
                        t3 = rtmp_pool.tile([128, S], f32, tag="t")
                        t4 = rtmp_pool.tile([128, S], f32, tag="t")
                        nc.vector.tensor_mul(t1, a, ct)
                        nc.vector.tensor_mul(t2, b, st)
                        nc.vector.tensor_sub(outT[:, g, :], t1, t2)
                        nc.vector.tensor_mul(t3, b, ct)
                        nc.vector.tensor_mul(t4, a, st)
                        nc.vector.tensor_add(outT[:, g + 4, :], t3, t4)

            # ---- v projection (token-major) ----
            with (
                tc.tile_pool(name="xv", bufs=1) as xin_pool,
                tc.tile_pool(name="wv", bufs=12) as w_pool,
                tc.tile_pool(name="psv", bufs=8, space="PSUM") as ps_pool,
            ):
                xT_s = xin_pool.tile([128, 32, S], bf16, tag="xT")
                xr = xvT_d[:].rearrange("(db p) t -> p db t", p=128)
                for c in range(4):
                    nc.sync.dma_start(
                        out=xT_s[:, c * 8 : (c + 1) * 8, :],
                        in_=xr[:, c * 8 : (c + 1) * 8, :],
                    )
                for fc in range(8):
                    pss = [ps_pool.tile([128, 512], f32, tag="ps", name="ps_v") for _ in range(4)]
                    for d in range(32):
                        wt = w_pool.tile([128, 512], bf16, tag="w")
                        nc.sync.dma_start(
                            out=wt,
                            in_=wvT_d[
                                d * 128 : (d + 1) * 128, fc * 512 : (fc + 1) * 512
                            ],
                        )
                        for tb in range(4):
                            nc.tensor.matmul(
                                pss[tb],
                                xT_s[:, d, tb * 128 : (tb + 1) * 128],
                                wt,
                                start=(d == 0),
                                stop=(d == 31),
                            )
                    for tb in range(4):
                        nc.vector.tensor_copy(
                            v_s[:, tb, fc * 4 : (fc + 1) * 4, :], pss[tb]
                        )

            # ---- attention (per head) + output projection ----
            with tc.tile_pool(name="attn", bufs=1) as attn_pool:
                attnT_s = attn_pool.tile([128, G, S], bf16, tag="attnT")
                with (
                    tc.tile_pool(name="wexp", bufs=6) as wexp_pool,
                    tc.tile_pool(name="rsm", bufs=4) as rpool,
                    tc.tile_pool(name="ps_s", bufs=2, space="PSUM") as ps_s_pool,
                    tc.tile_pool(name="ps_o", bufs=2, space="PSUM") as ps_o_pool,
                    tc.tile_pool(name="ps_m", bufs=2, space="PSUM") as ps_m_pool,
                    tc.tile_pool(name="ps_r", bufs=2, space="PSUM") as ps_r_pool,
                ):
                    for g in range(G):
                        ps_o = ps_o_pool.tile([128, S], f32, tag="o")
                        ps_sum = ps_m_pool.tile([1, S], f32, tag="sum")
                        for kb in range(4):
                            ps_sc = ps_s_pool.tile([128, S], f32, tag="s")
                            nc.tensor.matmul(
                                ps_sc,
                                kT_s[:, g, kb * 128 : (kb + 1) * 128],
                                qT_s[:, g, :],
                                start=True,
                                stop=True,
                            )
                            wb = wexp_pool.tile([128, S], bf16, tag="w")
                            nc.scalar.activation(
                                wb, ps_sc, AF.Exp, bias=maskb[:, kb : kb + 1], scale=1.0
                            )
                            nc.tensor.matmul(
                                ps_o,
                                v_s[:, kb, g, :],
                                wb,
                                start=(kb == 0),
                                stop=(kb == 3),
                            )
                            nc.tensor.matmul(
                                ps_sum,
                                ones_bf,
                                wb,
                                start=(kb == 0),
                                stop=(kb == 3),
                            )
                        r = rpool.tile([1, S], f32, tag="r")
                        nc.vector.reciprocal(r, ps_sum)
                        ps_rb = ps_r_pool.tile([128, S], f32, tag="rb")
                        nc.tensor.matmul(ps_rb, ones_f1, r, start=True, stop=True)
                        # DVE can read only one PSUM operand per op: bounce
                        # the broadcast reciprocal through SBUF first.
                        rb_s = rpool.tile([128, S], f32, tag="rb_s")
                        nc.scalar.copy(rb_s, ps_rb)
                        nc.vector.tensor_mul(attnT_s[:, g, :], ps_o, rb_s)

                # ---- y = attn @ Wo.T  (token-major output) ----
                with (
                    tc.tile_pool(name="wo", bufs=12) as wo_pool,
                    tc.tile_pool(name="psy", bufs=8, space="PSUM") as psy_pool,
                    tc.tile_pool(name="yout", bufs=4) as y_pool,
                ):
                    for fc in range(8):
                        pss = [
                            psy_pool.tile([128, 512], f32, tag="ps", name="ps_y") for _ in range(4)
                        ]
                        for dD in range(32):
                            wt = wo_pool.tile([128, 512], bf16, tag="w")
                            nc.sync.dma_start(
                                out=wt,
                                in_=woT_d[
                                    dD * 128 : (dD + 1) * 128,
                                    fc * 512 : (fc + 1) * 512,
                                ],
                            )
                            for tb in range(4):
                                nc.tensor.matmul(
                                    pss[tb],
                                    attnT_s[:, dD, tb * 128 : (tb + 1) * 128],
                                    wt,
                                    start=(dD == 0),
                                    stop=(dD == 31),
                                )
                        for tb in range(4):
                            yt = y_pool.tile([128, 512], bf16, tag="y")
                            nc.vector.tensor_copy(yt, pss[tb])
                            nc.sync.dma_start(
                                out=y_d[
                                    tb * 128 : (tb + 1) * 128,
                                    fc * 512 : (fc + 1) * 512,
                                ],
                                in_=yt,
                            )


_NC_CACHE = None


def _get_program():
    global _NC_CACHE
    if _NC_CACHE is None:
        _NC_CACHE = build_program()
    return _NC_CACHE


def make_in_maps(query, key, value, mask, position_ids, Wq, Wk, Wv, Wo):
    bf16 = ml_dtypes.bfloat16
    WSH = D // NCORES

    def t_bf16(a):  # [m,n] fp32 -> [n,m] bf16, contiguous
        return np.asarray(a, np.float32).T.astype(bf16)

    wqT = t_bf16(np.asarray(Wq))
    wkT = t_bf16(np.asarray(Wk))
    wvT = t_bf16(np.asarray(Wv))
    woT = t_bf16(np.asarray(Wo))
    invf = (10000.0 ** (-np.arange(0, RD, 2, dtype=np.float32) / RD)).astype(
        np.float32
    )

    in_maps = []
    for b in range(NCORES):
        sl = slice(b * WSH, (b + 1) * WSH)
        in_maps.append(
            {
                "xqT": t_bf16(query[b]),
                "xkT": t_bf16(key[b]),
                "xvT": t_bf16(value[b]),
                # core b ships only its row-shard; the kernel AllGathers
                "wq_sh": np.ascontiguousarray(wqT[sl]),
                "wk_sh": np.ascontiguousarray(wkT[sl]),
                "wv_sh": np.ascontiguousarray(wvT[sl]),
                "wo_sh": np.ascontiguousarray(woT[sl]),
                "pos": np.ascontiguousarray(
                    np.asarray(position_ids[b], np.float32)
                ),
                "invf": invf,
                "maskin": np.ascontiguousarray(np.asarray(mask[b], np.int32)),
            }
        )
    return in_maps


def kernel(query, key, value, mask, position_ids, Wq, Wk, Wv, Wo):
    global LAST_RESULT
    nc = _get_program()
    in_maps = make_in_maps(
        query, key, value, mask, position_ids, Wq, Wk, Wv, Wo
    )
    res = run_bass_kernel_spmd(
        nc, in_maps, core_ids=list(range(NCORES)), trace=TRACE
    )
    LAST_RESULT = res
    out = np.stack([res.results[b]["y"] for b in range(NCORES)], axis=0)
    # y comes back bf16 (halves the d2h bytes); widen host-side.
    return np.ascontiguousarray(out.astype(np.float32))



# revision 11
# speedup vs baseline: 1.0725x; 1.0725x over previous
"""Grouped-query attention (B=8,S=512,D=4096,G=32) on 8 trn2 cores.

Strategy: data-parallel over the batch dim — core b handles batch b.
Per core, everything is computed in a feature-major ("transposed")
layout so no on-device transposes are needed:

  q^T[f,t] = sum_d WqT[d,f] * xqT[d,t]        (lhsT=WqT tile, rhs=xqT)
  k^T      likewise;  v[t,f] uses lhsT=xqT tile, rhs=WvT tile
  RoPE on q^T/k^T heads 0..7 (per-token angle, head g pairs with g+4)
  s^T[k,q] = kh^T_blk.T @ qh^T   (per head, 4 k-blocks of 128)
  w^T      = exp(s^T + maskbias) (no max-subtraction; logits are O(10))
  o^T[dh,q]= sum_kb vh_blk.T @ w^T_blk        (lhsT=vh block)
  sum[1,q] = ones.T @ w^T  -> r = 1/sum -> broadcast via rank-1 matmul
  attn^T   = o^T * r_bcast  (bf16)
  y[t,f]   = sum_D attnT_blk.T @ WoT tile

Matmuls run in bf16 (fp32 PSUM accumulation); softmax math in fp32.
Host side only shards, transposes (layout), casts dtypes and gathers.
"""

import math

import numpy as np
import ml_dtypes

import concourse.bass as bass
import concourse.mybir as mybir
import concourse.tile as tile
from concourse import bacc
from concourse.bass_utils import run_bass_kernel_spmd

B, S, D = 8, 512, 4096
G, DH = 32, 128
RD = 1024
ALPHA = 1.0 / math.sqrt(DH)
PI = math.pi
NCORES = 8
DT = mybir.dt
AF = mybir.ActivationFunctionType
ALU = mybir.AluOpType

# set by test.py to capture a profile
TRACE = False
LAST_RESULT = None


def _range_reduce(nc, ang, mtmp):
    """In-place reduce ang (>=0, < ~7*pi/2) into (-pi, pi] mod 2*pi."""
    for _ in range(3):
        # mtmp = (ang > pi) * 2pi ; ang -= mtmp
        nc.vector.tensor_scalar(mtmp, ang, PI, 2.0 * PI, ALU.is_gt, ALU.mult)
        nc.vector.tensor_sub(ang, ang, mtmp)


def build_program():
    # Bacc (not plain Bass): its compile pipeline lowers multi-sem waits to
    # the single ISA wait slot; plain Bass BIR fails walrus codegen.
    nc = bacc.Bacc(
        "TRN2", target_bir_lowering=False, debug=False, num_devices=NCORES
    )
    bf16 = DT.bfloat16
    f32 = DT.float32

    # Weights arrive SHARDED: core c holds rows [c*512, (c+1)*512) of each
    # transposed weight (1/8 of the bytes), packed host-side as
    # [8 gq][512 rows][512 cols] so a per-gq column slice is contiguous.
    # On-chip chunked AllGathers rebuild the full matrices in internal
    # DRAM — host->device traffic for weights drops 8x vs replication,
    # and chunking lets the first projection start after the first small
    # gather instead of a full-matrix one.
    WSH = D // NCORES  # 512 rows per shard
    xqT_d = nc.declare_dram_parameter("xqT", [D, S], bf16, isOutput=False)
    xkT_d = nc.declare_dram_parameter("xkT", [D, S], bf16, isOutput=False)
    xvT_d = nc.declare_dram_parameter("xvT", [D, S], bf16, isOutput=False)
    wq_sh = nc.declare_dram_parameter("wq_sh", [8, WSH, 512], bf16, isOutput=False)
    wk_sh = nc.declare_dram_parameter("wk_sh", [8, WSH, 512], bf16, isOutput=False)
    wv_sh = nc.declare_dram_parameter("wv_sh", [8, WSH, 512], bf16, isOutput=False)
    wo_sh = nc.declare_dram_parameter("wo_sh", [8, WSH, 512], bf16, isOutput=False)
    pos_d = nc.declare_dram_parameter("pos", [S], f32, isOutput=False)
    invf_d = nc.declare_dram_parameter("invf", [S], f32, isOutput=False)
    mask_d = nc.declare_dram_parameter("maskin", [S], DT.int32, isOutput=False)
    y_d = nc.declare_dram_parameter("y", [S, D], bf16, isOutput=True)

    RG = [list(range(NCORES))]

    with tile.TileContext(nc) as tc:
        with tc.tile_pool(name="dram", bufs=1, space="DRAM") as dram:
            # chunk sizes (in gq blocks of 512 cols) per weight: finest for
            # Wq (gates the first matmuls), coarser later to save CC floors.
            gathered = {}  # (which, gq) -> (tile, local_j)

            def gather_weight(which, sh_d, chunks):
                gq0 = 0
                for ci, csz in enumerate(chunks):
                    bounce = dram.tile(
                        [csz, WSH, 512], bf16,
                        tag=f"b{which}{ci}", name=f"bounce_{which}{ci}",
                    )
                    g_t = dram.tile(
                        [NCORES, csz, WSH, 512], bf16, addr_space="Shared",
                        tag=f"g{which}{ci}", name=f"gath_{which}{ci}",
                    )
                    nc.gpsimd.dma_start(
                        out=bounce, in_=sh_d[gq0 : gq0 + csz]
                    )
                    nc.gpsimd.collective_compute(
                        "AllGather",
                        ALU.bypass,
                        replica_groups=RG,
                        ins=[bounce.opt()],
                        outs=[g_t.opt()],
                    )
                    for j in range(csz):
                        gathered[(which, gq0 + j)] = (g_t, j)
                    gq0 += csz

            gather_weight("q", wq_sh, (1, 1, 2, 2, 2))
            gather_weight("k", wk_sh, (2, 2, 4))
            gather_weight("v", wv_sh, (4, 4))
            gather_weight("o", wo_sh, (8,))

            def w_slice(which, gq, db):
                """[128, 512] tile of wT[db*128:(db+1)*128, gq*512:(gq+1)*512]."""
                g_t, j = gathered[(which, gq)]
                r, lb = db // 4, db % 4
                return g_t[r, j, lb * 128 : (lb + 1) * 128, :]

            _build_body(nc, tc, xqT_d, xkT_d, xvT_d, w_slice,
                        pos_d, invf_d, mask_d, y_d)
    # Bacc lowering: splits multi-sem waits into the single ISA wait slot,
    # allocates registers, fuses nops. Required before walrus codegen.
    nc.compile()
    return nc


def _build_body(nc, tc, xqT_d, xkT_d, xvT_d, w_slice,
                pos_d, invf_d, mask_d, y_d):
    bf16 = DT.bfloat16
    f32 = DT.float32
    if True:
        with tc.tile_pool(name="persist", bufs=1) as persist:
            # trig tiles, broadcast over partitions: [128, S]
            sin_t = persist.tile([128, S], f32, tag="sin")
            cos_t = persist.tile([128, S], f32, tag="cos")
            sinq_t = persist.tile([128, S], f32, tag="sinq")
            cosq_t = persist.tile([128, S], f32, tag="cosq")
            maskb = persist.tile([128, 4], f32, tag="maskb")
            ones_bf = persist.tile([128, 1], bf16, tag="ones_bf")
            ones_f1 = persist.tile([1, 128], f32, tag="ones_f1")
            qT_s = persist.tile([128, G, S], bf16, tag="qT")
            kT_s = persist.tile([128, G, S], bf16, tag="kT")
            v_s = persist.tile([128, 4, G, DH], bf16, tag="v")

            nc.vector.memset(ones_bf, 1.0)
            nc.vector.memset(ones_f1, 1.0)

            # ---- setup: trig + mask ----
            # Trig is computed on one partition, then broadcast to all 128
            # partitions with a rank-1 matmul (ones [1,128] x row [1,S]).
            with (
                tc.tile_pool(name="setup", bufs=1) as setup,
                tc.tile_pool(name="ps_setup", bufs=2, space="PSUM") as ps_setup,
            ):
                pos1 = setup.tile([1, S], f32, tag="pos1")
                invf1 = setup.tile([1, S], f32, tag="invf1")
                angc = setup.tile([1, S], f32, tag="angc")
                mtmp = setup.tile([1, S], f32, tag="mtmp")
                mi = setup.tile([128, 4], DT.int32, tag="mi")
                mf = setup.tile([128, 4], f32, tag="mf")

                nc.sync.dma_start(out=pos1, in_=pos_d[None, :])
                nc.sync.dma_start(out=invf1, in_=invf_d[None, :])
                # angles for sin, reuse pos1 as buffer for sin-angles
                angs = pos1
                nc.vector.tensor_mul(angs, pos1, invf1)
                nc.vector.tensor_scalar_add(angc, angs, PI / 2.0)
                _range_reduce(nc, angs, mtmp)
                _range_reduce(nc, angc, mtmp)
                nc.scalar.activation(angs, angs, AF.Sin)
                nc.scalar.activation(angc, angc, AF.Sin)
                ps_sin = ps_setup.tile([128, S], f32, tag="b", name="ps_sin")
                ps_cos = ps_setup.tile([128, S], f32, tag="b", name="ps_cos")
                nc.tensor.matmul(ps_sin, ones_f1, angs, start=True, stop=True)
                nc.tensor.matmul(ps_cos, ones_f1, angc, start=True, stop=True)
                nc.scalar.copy(sin_t, ps_sin)
                nc.scalar.copy(cos_t, ps_cos)
                nc.scalar.mul(sinq_t, ps_sin, ALPHA)
                nc.scalar.mul(cosq_t, ps_cos, ALPHA)

                nc.sync.dma_start(out=mi, in_=mask_d[:].rearrange("(b p) -> p b", p=128))
                nc.vector.tensor_copy(mf, mi)
                # maskb = m * 1e9 - 1e9  (0 where m==1, -1e9 where m==0)
                nc.vector.tensor_scalar(
                    maskb, mf, 1.0e9, 1.0e9, ALU.mult, ALU.subtract
                )

            # ---- q and k projections (output feature-major) + RoPE ----
            for which, xT_d, outT, ct, st in (
                ("q", xqT_d, qT_s, cosq_t, sinq_t),
                ("k", xkT_d, kT_s, cos_t, sin_t),
            ):
                scale = ALPHA if which == "q" else 1.0
                with (
                    tc.tile_pool(name=f"x{which}", bufs=1) as xin_pool,
                    tc.tile_pool(name=f"w{which}", bufs=20) as w_pool,
                    tc.tile_pool(name=f"ps{which}", bufs=8, space="PSUM") as ps_pool,
                    tc.tile_pool(name=f"rope{which}", bufs=1) as rope_pool,
                    tc.tile_pool(name=f"rt{which}", bufs=8) as rtmp_pool,
                ):
                    xT_s = xin_pool.tile([128, 32, S], bf16, tag="xT")
                    xr = xT_d[:].rearrange("(db p) t -> p db t", p=128)
                    for c in range(4):
                        nc.sync.dma_start(
                            out=xT_s[:, c * 8 : (c + 1) * 8, :],
                            in_=xr[:, c * 8 : (c + 1) * 8, :],
                        )
                    rope_f32 = rope_pool.tile([128, 8, S], f32, tag="rope")
                    for gq in range(8):
                        pss = [
                            ps_pool.tile([128, S], f32, tag="ps", name="ps_qk") for _ in range(4)
                        ]
                        for d in range(32):
                            wt = w_pool.tile([128, 512], bf16, tag="w")
                            nc.sync.dma_start(
                                out=wt,
                                in_=w_slice(which, gq, d),
                            )
                            for g4 in range(4):
                                nc.tensor.matmul(
                                    pss[g4],
                                    wt[:, g4 * 128 : (g4 + 1) * 128],
                                    xT_s[:, d, :],
                                    start=(d == 0),
                                    stop=(d == 31),
                                )
                        for g4 in range(4):
                            g = gq * 4 + g4
                            if g < 8:
                                # RoPE heads: stash fp32
                                nc.vector.tensor_copy(rope_f32[:, g, :], pss[g4])
                            elif which == "q":
                                nc.scalar.activation(
                                    outT[:, g, :], pss[g4], AF.Copy, scale=scale
                                )
                            else:
                                nc.vector.tensor_copy(outT[:, g, :], pss[g4])
                    # RoPE: head g pairs with head g+4 (per-token scalar angle)
                    for g in range(4):
                        a = rope_f32[:, g, :]
                        b = rope_f32[:, g + 4, :]
                        t1 = rtmp_pool.tile([128, S], f32, tag="t")
                        t2 = rtmp_pool.tile([128, S], f32, tag="t")
                        t3 = rtmp_pool.tile([128, S], f32, tag="t")
                        t4 = rtmp_pool.tile([128, S], f32, tag="t")
                        nc.vector.tensor_mul(t1, a, ct)
                        nc.vector.tensor_mul(t2, b, st)
                        nc.vector.tensor_sub(outT[:, g, :], t1, t2)
                        nc.vector.tensor_mul(t3, b, ct)
                        nc.vector.tensor_mul(t4, a, st)
                        nc.vector.tensor_add(outT[:, g + 4, :], t3, t4)

            # ---- v projection (token-major) ----
            with (
                tc.tile_pool(name="xv", bufs=1) as xin_pool,
                tc.tile_pool(name="wv", bufs=20) as w_pool,
                tc.tile_pool(name="psv", bufs=8, space="PSUM") as ps_pool,
            ):
                xT_s = xin_pool.tile([128, 32, S], bf16, tag="xT")
                xr = xvT_d[:].rearrange("(db p) t -> p db t", p=128)
                for c in range(4):
                    nc.sync.dma_start(
                        out=xT_s[:, c * 8 : (c + 1) * 8, :],
                        in_=xr[:, c * 8 : (c + 1) * 8, :],
                    )
                for fc in range(8):
                    pss = [ps_pool.tile([128, 512], f32, tag="ps", name="ps_v") for _ in range(4)]
                    for d in range(32):
                        wt = w_pool.tile([128, 512], bf16, tag="w")
                        nc.sync.dma_start(
                            out=wt,
                            in_=w_slice("v", fc, d),
                        )
                        for tb in range(4):
                            nc.tensor.matmul(
                                pss[tb],
                                xT_s[:, d, tb * 128 : (tb + 1) * 128],
                                wt,
                                start=(d == 0),
                                stop=(d == 31),
                            )
                    for tb in range(4):
                        nc.vector.tensor_copy(
                            v_s[:, tb, fc * 4 : (fc + 1) * 4, :], pss[tb]
                        )

            # ---- attention (per head) + output projection ----
            with tc.tile_pool(name="attn", bufs=1) as attn_pool:
                attnT_s = attn_pool.tile([128, G, S], bf16, tag="attnT")
                with (
                    tc.tile_pool(name="wexp", bufs=6) as wexp_pool,
                    tc.tile_pool(name="rsm", bufs=4) as rpool,
                    tc.tile_pool(name="ps_s", bufs=2, space="PSUM") as ps_s_pool,
                    tc.tile_pool(name="ps_o", bufs=2, space="PSUM") as ps_o_pool,
                    tc.tile_pool(name="ps_m", bufs=2, space="PSUM") as ps_m_pool,
                    tc.tile_pool(name="ps_r", bufs=2, space="PSUM") as ps_r_pool,
                ):
                    for g in range(G):
                        ps_o = ps_o_pool.tile([128, S], f32, tag="o")
                        ps_sum = ps_m_pool.tile([1, S], f32, tag="sum")
                        for kb in range(4):
                            ps_sc = ps_s_pool.tile([128, S], f32, tag="s")
                            nc.tensor.matmul(
                                ps_sc,
                                kT_s[:, g, kb * 128 : (kb + 1) * 128],
                                qT_s[:, g, :],
                                start=True,
                                stop=True,
                            )
                            wb = wexp_pool.tile([128, S], bf16, tag="w")
                            nc.scalar.activation(
                                wb, ps_sc, AF.Exp, bias=maskb[:, kb : kb + 1], scale=1.0
                            )
                            nc.tensor.matmul(
                                ps_o,
                                v_s[:, kb, g, :],
                                wb,
                                start=(kb == 0),
                                stop=(kb == 3),
                            )
                            nc.tensor.matmul(
                                ps_sum,
                                ones_bf,
                                wb,
                                start=(kb == 0),
                                stop=(kb == 3),
                            )
                        r = rpool.tile([1, S], f32, tag="r")
                        nc.vector.reciprocal(r, ps_sum)
                        ps_rb = ps_r_pool.tile([128, S], f32, tag="rb")
                        nc.tensor.matmul(ps_rb, ones_f1, r, start=True, stop=True)
                        # DVE can read only one PSUM operand per op: bounce
                        # the broadcast reciprocal through SBUF first.
                        rb_s = rpool.tile([128, S], f32, tag="rb_s")
                        nc.scalar.copy(rb_s, ps_rb)
                        nc.vector.tensor_mul(attnT_s[:, g, :], ps_o, rb_s)

                # ---- y = attn @ Wo.T  (token-major output) ----
                with (
                    tc.tile_pool(name="wo", bufs=20) as wo_pool,
                    tc.tile_pool(name="psy", bufs=8, space="PSUM") as psy_pool,
                    tc.tile_pool(name="yout", bufs=4) as y_pool,
                ):
                    for fc in range(8):
                        pss = [
                            psy_pool.tile([128, 512], f32, tag="ps", name="ps_y") for _ in range(4)
                        ]
                        for dD in range(32):
                            wt = wo_pool.tile([128, 512], bf16, tag="w")
                            nc.sync.dma_start(
                                out=wt,
                                in_=w_slice("o", fc, dD),
                            )
                            for tb in range(4):
                                nc.tensor.matmul(
                                    pss[tb],
                                    attnT_s[:, dD, tb * 128 : (tb + 1) * 128],
                                    wt,
                                    start=(dD == 0),
                                    stop=(dD == 31),
                                )
                        for tb in range(4):
                            yt = y_pool.tile([128, 512], bf16, tag="y")
                            nc.vector.tensor_copy(yt, pss[tb])
                            nc.sync.dma_start(
                                out=y_d[
                                    tb * 128 : (tb + 1) * 128,
                                    fc * 512 : (fc + 1) * 512,
                                ],
                                in_=yt,
                            )


_NC_CACHE = None


def _get_program():
    global _NC_CACHE
    if _NC_CACHE is None:
        _NC_CACHE = build_program()
    return _NC_CACHE


def make_in_maps(query, key, value, mask, position_ids, Wq, Wk, Wv, Wo):
    bf16 = ml_dtypes.bfloat16
    WSH = D // NCORES

    def t_bf16(a):  # [m,n] fp32 -> [n,m] bf16, contiguous
        return np.asarray(a, np.float32).T.astype(bf16)

    wqT = t_bf16(np.asarray(Wq))
    wkT = t_bf16(np.asarray(Wk))
    wvT = t_bf16(np.asarray(Wv))
    woT = t_bf16(np.asarray(Wo))
    invf = (10000.0 ** (-np.arange(0, RD, 2, dtype=np.float32) / RD)).astype(
        np.float32
    )

    def shard(wT, b):
        # rows [b*512,(b+1)*512), packed [8 gq][512 rows][512 cols] so each
        # per-gq column slice is one contiguous AllGather input chunk
        s = wT[b * WSH : (b + 1) * WSH]  # [512, 4096]
        return np.ascontiguousarray(s.reshape(WSH, 8, 512).transpose(1, 0, 2))

    in_maps = []
    for b in range(NCORES):
        in_maps.append(
            {
                "xqT": t_bf16(query[b]),
                "xkT": t_bf16(key[b]),
                "xvT": t_bf16(value[b]),
                # core b ships only its row-shard; the kernel AllGathers
                "wq_sh": shard(wqT, b),
                "wk_sh": shard(wkT, b),
                "wv_sh": shard(wvT, b),
                "wo_sh": shard(woT, b),
                "pos": np.ascontiguousarray(
                    np.asarray(position_ids[b], np.float32)
                ),
                "invf": invf,
                "maskin": np.ascontiguousarray(np.asarray(mask[b], np.int32)),
            }
        )
    return in_maps


def kernel(query, key, value, mask, position_ids, Wq, Wk, Wv, Wo):
    global LAST_RESULT
    nc = _get_program()
    in_maps = make_in_maps(
        query, key, value, mask, position_ids, Wq, Wk, Wv, Wo
    )
    res = run_bass_kernel_spmd(
        nc, in_maps, core_ids=list(range(NCORES)), trace=TRACE
    )
    LAST_RESULT = res
    out = np.stack([res.results[b]["y"] for b in range(NCORES)], axis=0)
    # y comes back bf16 (halves the d2h bytes); widen host-side.
    return np.ascontiguousarray(out.astype(np.float32))

